# revision 3
# baseline (speedup 1.0000x reference)
"""Trainium2 Bass kernel for: ConvTranspose3d(16->64, k=4, s=2, p=1) + conv_bias,
mean over depth, + bias, channel softmax, tanh, *2.

Input  x: (16, 16, 16, 32, 32) f32  -> Output: (16, 64, 1, 64, 64) f32.

v2 design (fp16 end-to-end; the fp32r baseline was DVE-bound at ~96%):

  Depth-mean commutes with the transposed conv (see host_constants):
    mean_d' ConvT3D(x, w) = ConvT2D(A, W2) / 32,  A = [sum_d x, x[:,0], x[:,15]].

  * x is host-pre-transposed/cast to f16 [2, 128, 1024] per batch
    (partitions = (d-tile, c)), so no on-device rounding pass is needed and
    input DMA bytes halve.
  * Selector matmul forms A in PSUM (2 accumulating f16 matmuls, N=1024).
  * B stack [128, 34, 34] f16: block0 = A padded (ACT copy from PSUM),
    block1 = block0 shifted one row down (DVE 4x-packed f16 copy).
  * Conv: both h-parities share one rhs stream. For window rows v=2..33 of B,
    out partitions 0:64 get ph0 weights (kh=1,3), partitions 64:128 get ph1
    weights (kh=0,2); column j of the stream is output m=v-1 (ph0) / m=v-2
    (ph1). 2 w-taps accumulate in PSUM => 2 matmuls of N=1024 per w-parity.
    Boundary: ph0 m=0 needs one extra 2-matmul pair on B row 1 (N=32); ph1 is
    fully covered (B rows 0 and 33 are zero = A[-1]/A[32]).
  * E4 [128, 33, 32, 2] f16: exp(conv+bias) lands at rows 1:33 with the
    w-parity interleaved by the ACT write AP, so every later free-dim walk is
    contiguous f16 (DVE 2x/4x packing) and the store rows are w'-ordered.
    Row 0: ph0 half = fixup exp, ph1 half = const 1.0 (denominator dummy).
  * Softmax sums: ones-block lhsT [128,128] replicates each half's channel
    sum across that half's 64 partitions => no separate broadcast matmul.
    reciprocal_approx_fast (DVE) -> R4 f16; numerator multiply is one f16
    2x-packed DVE op; tanh on ACT (exp and tanh share act-table set 0 =>
    no table reloads); *2 is one 4x-packed DVE tensor-scalar.
  * Stores: 2 per batch (h'-parity halves), f16, w' already interleaved.

Sharding: data-parallel over batch, 2 batches per core on 8 cores.
"""

import numpy as np

import concourse.bacc as bacc
import concourse.mybir as mybir
import concourse.tile as tile
from concourse.bass_utils import run_bass_kernel_spmd

B_TOTAL = 16
IN_C, OUT_C = 16, 64
D_IN, H_IN, W_IN = 16, 32, 32
KK, STRIDE, PAD = 4, 2, 1
SCALE = 2.0
D_OUT = 32  # conv output depth (before mean)
N_CORES = 8
B_LOC = B_TOTAL // N_CORES

F32 = mybir.dt.float32
F16 = mybir.dt.float16

AF = mybir.ActivationFunctionType

# A/B experiment knobs (HW per-instruction overhead is ~2x the sim's, so
# instruction/hop count trades against engine busy-time; tuned empirically)
B0_MODE = "act"     # "split" | "act" | "dve"  (psA -> B block0 copy)
B1_SRC = "b0"       # "b0" | "psA"             (block1 source)
MUL_MODE = "split"  # "split" | "dve" | "pool" (softmax numerator multiply)


def build_bass(repeat=1):
    """repeat>1 re-runs the whole per-core workload in one NEFF (for timing:
    wall(L) - wall(1) isolates device time from dispatch overhead)."""
    nc = bacc.Bacc(name="deconv_mean_softmax_v2")

    x_d = nc.dram_tensor("x", [128, B_LOC * 2 * 1024], F16, kind="ExternalInput")
    wsel_d = nc.dram_tensor("wsel", [128, 2, 48], F16, kind="ExternalInput")
    wk2_d = nc.dram_tensor("wk2", [112, 4, 128], F16, kind="ExternalInput")
    ones_d = nc.dram_tensor("ones2", [128, 128], F16, kind="ExternalInput")
    bias_d = nc.dram_tensor("bias2", [128, 1], F32, kind="ExternalInput")
    einit_d = nc.dram_tensor("einit", [128, 33 * 64], F16, kind="ExternalInput")
    bz_d = nc.dram_tensor("bzero", [128, 34 * 34], F16, kind="ExternalInput")
    # ph-blocked scratch layout: 4KB contiguous per partition per store (the
    # h-parity interleave happens on the host during unshard)
    out_d = nc.dram_tensor("out", [B_LOC, 2, OUT_C, 32, 64], F16, kind="ExternalOutput")

    NB = 3   # B slots
    NE = 4   # E slots (written S1(k), still read at S3(k))

    with tile.TileContext(nc) as tc:
        with (
            tc.tile_pool(name="consts", bufs=1) as consts,
            tc.tile_pool(name="xin", bufs=3) as xin,
            tc.tile_pool(name="rpool", bufs=3) as rpool,
            tc.tile_pool(name="opool", bufs=3) as opool,
            tc.tile_pool(name="psum_big", bufs=3, space="PSUM") as psum_big,
            tc.tile_pool(name="psum_sm", bufs=2, space="PSUM") as psum_sm,

        ):
            wsel = consts.tile([128, 2, 48], F16)
            nc.sync.dma_start(out=wsel, in_=wsel_d[:, :, :])
            wk2 = consts.tile([112, 4, 128], F16)
            nc.sync.dma_start(out=wk2, in_=wk2_d[:, :, :])
            ones2 = consts.tile([128, 128], F16)
            nc.sync.dma_start(out=ones2, in_=ones_d[:, :])
            bias2 = consts.tile([128, 1], F32)
            nc.sync.dma_start(out=bias2, in_=bias_d[:, :])

            B_slots = []
            E_slots = []
            for i in range(max(NB, NE)):
                if i < NB:
                    bs = consts.tile([128, 34, 34], F16, tag=f"Bslot{i}")
                    nc.sync.dma_start(
                        out=bs.rearrange("p a b -> p (a b)"), in_=bz_d[:, :]
                    )
                    B_slots.append(bs)
                if i < NE:
                    es = consts.tile([128, 33, 32, 2], F16, tag=f"Eslot{i}")
                    nc.sync.dma_start(
                        out=es.rearrange("p a b c -> p (a b c)"), in_=einit_d[:, :]
                    )
                    E_slots.append(es)

            # ---- software pipeline over batch-ticks k = rep*B_LOC + b ----
            # SX(k): x prefetch | S0(k): sel+B | S1(k): conv+exp | S2(k):
            # sums+recip | S3(k): mul | S4(k): tanh+scale+store.  Tick t
            # emits S4(t-4) S3(t-3) S2(t-2) S1(t-1) S0(t) SX(t+1): in-order
            # engines then always have a previous-batch stage to run while a
            # dependency drains, so the steady state is engine-bound.
            total = repeat * B_LOC
            xts, Es, Rs, Os = {}, {}, {}, {}

            def SX(k):
                # one DMA fetches the whole iteration's x (all local batches)
                if k % B_LOC == 0:
                    xp = xin.tile([128, B_LOC, 2, 1024], F16, tag="xt")
                    nc.sync.dma_start(
                        out=xp.rearrange("p a b c -> p (a b c)"), in_=x_d[:, :]
                    )
                    for j in range(B_LOC):
                        xts[k + j] = xp[:, j]

            def S0(k):
                xt = xts.pop(k)
                psA = psum_big.tile([48, 1024], F32, tag="big")
                for h in range(2):
                    for t in range(2):
                        nc.tensor.matmul(
                            psA[:, h * 512 : (h + 1) * 512],
                            wsel[:, t, :],
                            xt[:, t, h * 512 : (h + 1) * 512],
                            start=(t == 0), stop=(t == 1),
                        )
                psA3 = psA.rearrange("p (h w) -> p h w", w=32)
                Bt = B_slots[k % NB]
                if B0_MODE == "split":
                    nc.scalar.copy(out=Bt[0:48, 1:13, 1:33], in_=psA3[:, 0:12, :])
                    nc.vector.tensor_copy(
                        out=Bt[0:48, 13:33, 1:33], in_=psA3[:, 12:32, :]
                    )
                elif B0_MODE == "split2":
                    nc.scalar.copy(out=Bt[0:48, 1:21, 1:33], in_=psA3[:, 0:20, :])
                    nc.vector.tensor_copy(
                        out=Bt[0:48, 21:33, 1:33], in_=psA3[:, 20:32, :]
                    )
                elif B0_MODE == "act":
                    nc.scalar.copy(out=Bt[0:48, 1:33, 1:33], in_=psA3)
                elif B0_MODE == "dma":
                    # casting SWDGE DMA: PSUM fp32 -> SBUF f16 on a DMA
                    # engine; frees both ACT and DVE of the block0 copy
                    nc.gpsimd.dma_start(out=Bt[0:48, 1:33, 1:33], in_=psA3)
                else:
                    nc.vector.tensor_copy(out=Bt[0:48, 1:33, 1:33], in_=psA3)
                if B1_SRC == "b0":
                    nc.vector.tensor_copy(
                        out=Bt[64:112, 2:34, :], in_=Bt[0:48, 1:33, :]
                    )
                else:
                    nc.vector.tensor_copy(
                        out=Bt[64:112, 2:34, 1:33], in_=psA3
                    )

            def S1(k):
                Bt = B_slots[k % NB]
                Et = E_slots[k % NE]
                for pw in (0, 1):
                    psC = psum_big.tile([128, 32, 32], F32, tag="big")
                    for w0 in (0, 1):
                        r0 = 2 + 16 * w0
                        for tap in (0, 1):
                            c0 = (1 + pw) - tap
                            nc.tensor.matmul(
                                psC[:, 16 * w0 : 16 * w0 + 16, :],
                                wk2[:, pw * 2 + tap, :],
                                Bt[0:112, r0 : r0 + 16, c0 : c0 + 32],
                                start=(tap == 0), stop=(tap == 1),
                            )
                    nc.scalar.activation(
                        out=Et[:, 1:33, :, pw], in_=psC,
                        func=AF.Exp, bias=bias2, scale=1.0,
                    )
                # ph0 m=0 fixup: B row 1 (block1 rows there are zero)
                psF = psum_sm.tile([64, 2, 32], F32, tag="small")
                for pw in (0, 1):
                    for tap in (0, 1):
                        c0 = (1 + pw) - tap
                        nc.tensor.matmul(
                            psF[:, pw, :],
                            wk2[:, pw * 2 + tap, 0:64],
                            Bt[0:112, 1, c0 : c0 + 32],
                            start=(tap == 0), stop=(tap == 1),
                        )
                nc.scalar.activation(
                    out=Et[0:64, 0, :, :],
                    in_=psF.rearrange("p pw w -> p w pw"),
                    func=AF.Exp, bias=bias2[0:64], scale=1.0,
                )
                Es[k] = Et

            def S2(k):
                Et = Es[k]
                psS1 = psum_big.tile([128, 1024], F32, tag="big")
                psS2 = psum_big.tile([128, 1024], F32, tag="big")
                for j, ps in ((0, psS1), (1, psS1), (2, psS2), (3, psS2)):
                    nc.tensor.matmul(
                        ps[:, (j % 2) * 512 : (j % 2) * 512 + 512],
                        ones2,
                        Et[:, 8 * j : 8 * j + 8, :, :].rearrange(
                            "p a b c -> p (a b c)"
                        ),
                    )
                psS3 = psum_sm.tile([128, 64], F32, tag="small")
                nc.tensor.matmul(
                    psS3, ones2, Et[:, 32, :, :].rearrange("p b c -> p (b c)")
                )
                Rt = rpool.tile([128, 33, 32, 2], F32, tag="R")
                nc.vector.reciprocal_approx_fast(
                    out=Rt[:, 0:16, :, :].rearrange("p a b c -> p (a b c)"), in_=psS1
                )
                nc.vector.reciprocal_approx_fast(
                    out=Rt[:, 16:32, :, :].rearrange("p a b c -> p (a b c)"), in_=psS2
                )
                nc.vector.reciprocal_approx_fast(
                    out=Rt[:, 32, :, :].rearrange("p b c -> p (b c)"), in_=psS3
                )
                Rs[k] = Rt

            def S3(k):
                Et = Es.pop(k)
                Rt = Rs.pop(k)
                Ot = opool.tile([128, 33 * 64], F16, tag="O")
                Ef = Et.rearrange("p a b c -> p (a b c)")
                Rf = Rt.rearrange("p a b c -> p (a b c)")
                if MUL_MODE == "split":
                    nc.gpsimd.tensor_mul(
                        Ot[:, 704:2112], Ef[:, 704:2112], Rf[:, 704:2112]
                    )
                    nc.vector.tensor_mul(Ot[:, 0:704], Ef[:, 0:704], Rf[:, 0:704])
                elif MUL_MODE == "dve":
                    nc.vector.tensor_mul(Ot, Ef, Rf)
                else:
                    nc.gpsimd.tensor_mul(Ot, Ef, Rf)
                Os[k] = Ot

            def S4(k):
                Ot = Os.pop(k)
                nc.scalar.activation(out=Ot, in_=Ot, func=AF.Tanh)
                nc.vector.tensor_scalar_mul(Ot, Ot, SCALE)
                O3 = Ot.rearrange("p (a bc) -> p a bc", bc=64)
                b = k % B_LOC
                nc.sync.dma_start(
                    out=out_d[b, 0].rearrange("c m w -> c (m w)"),
                    in_=O3[0:64, 0:32, :].rearrange("c m w -> c (m w)"),
                )
                nc.sync.dma_start(
                    out=out_d[b, 1].rearrange("c m w -> c (m w)"),
                    in_=O3[64:128, 1:33, :].rearrange("c m w -> c (m w)"),
                )

            for t in range(total + 5):
                if 0 <= t - 2 < total:
                    S2(t - 2)
                if 0 <= t - 4 < total:
                    S4(t - 4)
                if 0 <= t - 3 < total:
                    S3(t - 3)
                if 0 <= t - 1 < total:
                    S1(t - 1)
                if t < total:
                    if t == 0:
                        SX(0)
                    S0(t)
                if t + 2 < total and (t + 2) % B_LOC == 0:
                    SX(t + 2)

    return nc


def host_constants(weight, conv_bias, bias):
    w = np.asarray(weight, np.float32).astype(np.float64)
    W2 = np.empty((48, OUT_C, KK, KK), np.float64)
    W2[0:16] = w.sum(axis=2) / D_OUT
    W2[16:32] = -w[:, :, 0] / D_OUT
    W2[32:48] = -w[:, :, 3] / D_OUT

    # block0 <-> kh = KH[ph][0], block1 <-> kh = KH[ph][1]
    KH = {0: (1, 3), 1: (0, 2)}
    KW = {0: (1, 3), 1: (0, 2)}
    wk2 = np.zeros((112, 4, 128), np.float64)
    for pw in (0, 1):
        for tap in (0, 1):
            j = pw * 2 + tap
            kw = KW[pw][tap]
            for ph, col0 in ((0, 0), (1, 64)):
                wk2[0:48, j, col0 : col0 + 64] = W2[:, :, KH[ph][0], kw]
                wk2[64:112, j, col0 : col0 + 64] = W2[:, :, KH[ph][1], kw]

    # selector for A = [sum_d x, x[:,0], x[:,15]] per d-tile t
    wsel = np.zeros((128, 2, 48), np.float64)
    for t in range(2):
        for dd in range(8):
            d = t * 8 + dd
            for c in range(IN_C):
                p = dd * IN_C + c
                wsel[p, t, c] = 1.0
                if d == 0:
                    wsel[p, t, 16 + c] = 1.0
                if d == 15:
                    wsel[p, t, 32 + c] = 1.0

    bias_comb = (
        np.asarray(conv_bias, np.float64) + np.asarray(bias, np.float64).reshape(-1)
    )
    bias2 = np.tile(bias_comb, 2).reshape(128, 1)

    ones2 = np.zeros((128, 128), np.float16)
    ones2[0:64, 0:64] = 1.0
    ones2[64:128, 64:128] = 1.0

    einit = np.zeros((128, 33, 32, 2), np.float16)
    einit[64:128, 0, :, :] = 1.0  # ph1 row-0 denominator dummy

    return {
        "wsel": wsel.astype(np.float16),
        "wk2": wk2.astype(np.float16),
        "bias2": bias2.astype(np.float32),
        "ones2": ones2,
        "einit": einit.reshape(128, 33 * 64),
        "bzero": np.zeros((128, 34 * 34), np.float16),
    }


_CACHED = {}


def make_in_maps(inputs):
    x = np.asarray(inputs["x"], np.float32)
    consts = host_constants(inputs["weight"], inputs["conv_bias"], inputs["bias"])
    in_maps = []
    for core in range(N_CORES):
        xs = x[core * B_LOC : (core + 1) * B_LOC]
        # (b, c, d, h, w) -> (b, d, c, h*w) -> (b, 2, 128, 1024) f16
        xt = np.ascontiguousarray(xs.transpose(0, 2, 1, 3, 4)).reshape(
            B_LOC, 2, 128, 1024
        )
        xt = np.ascontiguousarray(xt.transpose(2, 0, 1, 3)).reshape(
            128, B_LOC * 2048
        )
        in_maps.append({"x": xt.astype(np.float16), **consts})
    return in_maps


def kernel(x, weight, conv_bias, bias):
    if "nc" not in _CACHED:
        nc = build_bass()
        nc.finalize()
        _CACHED["nc"] = nc
    nc = _CACHED["nc"]

    in_maps = make_in_maps(
        {"x": x, "weight": weight, "conv_bias": conv_bias, "bias": bias}
    )

    res = run_bass_kernel_spmd(nc, in_maps, core_ids=list(range(N_CORES)))
    outs = [r["out"] for r in res.results]
    scr = np.concatenate(outs, axis=0)  # (16, 2, 64, 32, 64) ph-blocked f16
    full = np.ascontiguousarray(scr.transpose(0, 2, 3, 1, 4)).reshape(
        B_TOTAL, OUT_C, 64, 64
    )
    return full.astype(np.float32)[:, :, None, :, :]


if __name__ == "__main__":
    import reference

    inputs = reference.setup_inputs()
    out = kernel(**{k: np.asarray(v) for k, v in inputs.items()})
    print("kernel out", out.shape, out.dtype)
